# revision 36
# baseline (speedup 1.0000x reference)
"""Trainium2 Bass kernel for nn_Attention_41085657153633.

B=8, N=1024, C=384, H=6, D=64 attention with per-head q/k normalization
(mean/std over head_dim, ddof=1), softmax(QK^T/sqrt(D)) @ V, output proj.

Sharding: data-parallel over B — each of the 8 NeuronCores handles one
batch element end-to-end (no collectives).

v4 design (ACT-bound pipeline, ~all-bf16):
  - all matmul operands bf16 (FWL weight loads, half DMA, 2x DVE modes).
  - scores per (t-chunk, head): the two heads of a pair run CONCURRENTLY
    on PE row groups 0/64; the g-stagger ping-pongs the two s tiles so
    exp(t,g0) overlaps QK(t,g1) and ACT (the bottleneck engine) stays fed.
  - softmax denominator via 64 ones-columns in the AV weights (free in PE
    streaming); 1/denom = exp(-ln d) on ACT (ln/exp share one table set).
  - PE filler work (next chunks' QKV+norm, V, previous pair's AV, proj
    partials) is spread round-robin between QK t-steps; keep-warm pad
    matmuls fill any slack so the HAM activity monitor never re-throttles
    the PE clock to 1.2GHz mid-kernel.
  - norm chain allocs both PSUM mean/ssq tiles after both qkr copies so
    no matmul ever waits an ACT-queued reader (wps pool is 2-deep).
  - proj: k=0,1 partials (+bias) staged to SBUF bf16 during pair 2; at
    the tail k=2 is split into two 64-row contraction halves (head 4
    right after its normalizer, head 5 after the last denominator), the
    staged partial is folded back via an identity matmul, and the
    PSUM->SBUF moves alternate ACT/DVE so the three output chunks drain
    in parallel with the DMA.
"""

import sys

sys.path.insert(0, "/opt/trn_rl_repo")

import json

import numpy as np

B, N, C = 8, 1024, 384
H, D = 6, 64
NCORES = 8
KC = C // 128   # 3 contraction chunks of the model dim
TC = N // 128   # 8 token chunks

_prog = None


def _install_multiwait_fixup():
    """This container's walrus build rejects >1 sync wait per instruction
    ("Too many sync wait commands"). Rewrite the BIR JSON before compile:
    hoist extra waits onto single-wait EventSemaphore instructions
    inserted just before the owner on the same engine (engines dispatch
    in program order, so the gating is preserved)."""
    from concourse import bass2jax, bass_utils

    if getattr(bass_utils, "_multiwait_fixup", False):
        return
    bass_utils._multiwait_fixup = True

    orig = bass_utils.compile_bir_kernel

    def _split(bir_json: bytes) -> bytes:
        j = json.loads(bir_json)
        for fn in j.get("functions", []):
            for bb in fn.get("blocks", []):
                out = []
                for inst in bb.get("instructions", []):
                    si = inst.get("sync_info")
                    waits = si.get("on_wait", []) if si else []
                    if len(waits) > 1:
                        for k, w in enumerate(waits[:-1]):
                            out.append({
                                "debug": inst.get("debug", 0),
                                "engine": inst["engine"],
                                "ins": [],
                                "outs": [],
                                "name": f"{inst['name']}-sw{k}",
                                "opcode": "EventSemaphore",
                                "sync_info": {"on_update": [], "on_wait": [w]},
                            })
                        si["on_wait"] = [waits[-1]]
                    out.append(inst)
                bb["instructions"] = out
        return json.dumps(j).encode()

    def patched(bir_json, tmpdir, neff_name="file.neff"):
        return orig(_split(bir_json), tmpdir, neff_name)

    bass_utils.compile_bir_kernel = patched
    bass2jax.compile_bir_kernel = patched


def _build():
    import concourse.bass as bass
    import concourse.tile as tile
    from concourse import mybir

    _install_multiwait_fixup()

    F32 = mybir.dt.float32
    BF16 = mybir.dt.bfloat16
    EXP = mybir.ActivationFunctionType.Exp
    LN = mybir.ActivationFunctionType.Ln

    nc = bass.Bass("TRN2")
    xT = nc.dram_tensor("xT", [C, N], BF16, kind="ExternalInput")
    qkv_wT = nc.dram_tensor("qkv_wT", [C, 3 * C], BF16, kind="ExternalInput")
    proj_wT = nc.dram_tensor("proj_wT", [C, C], BF16, kind="ExternalInput")
    pb = nc.dram_tensor("pb", [128, 3], F32, kind="ExternalInput")
    bd_in = nc.dram_tensor("bd", [128, 128], BF16, kind="ExternalInput")
    id_in = nc.dram_tensor("ident", [128, 128], BF16, kind="ExternalInput")
    outT = nc.dram_tensor("outT", [C, N], F32, kind="ExternalOutput")

    scale = float(D) ** -0.5

    with tile.TileContext(nc) as tc:
      with nc.allow_low_precision(reason="bf16 matmul/elementwise pipeline"):
        with tc.tile_pool(name="consts", bufs=1) as consts, \
             tc.tile_pool(name="ins", bufs=1) as ins, \
             tc.tile_pool(name="persist", bufs=1) as persist, \
             tc.tile_pool(name="wk", bufs=3) as wk, \
             tc.tile_pool(name="es", bufs=32) as esp, \
             tc.tile_pool(name="rec", bufs=3) as recp, \
             tc.tile_pool(name="po", bufs=2) as pop, \
             tc.tile_pool(name="sps", bufs=2, space="PSUM") as sps, \
             tc.tile_pool(name="avps", bufs=1, space="PSUM") as avps, \
             tc.tile_pool(name="wps", bufs=2, space="PSUM") as wps:

            # ---- consts + ACT table preload ----
            bd = consts.tile([128, 128], BF16)
            ident = consts.tile([128, 128], BF16)
            pbt = consts.tile([128, 3], F32)
            ddof_b = consts.tile([128, 1], F32)
            nc.vector.memset(ddof_b[:], -0.5 * float(np.log(64.0 / 63.0)))
            dum = consts.tile([1, 16], F32)
            nc.vector.memset(dum[:], 1.0)
            dum2 = consts.tile([1, 16], F32)
            # force the natural_log_exp table set resident during DMA wait
            nc.scalar.activation(dum2[:], dum[:], LN)
            nc.scalar.activation(dum2[:], dum[:], EXP)

            nc.sync.dma_start(out=bd[:], in_=bd_in[:, :])
            nc.sync.dma_start(out=pbt[:], in_=pb[:, :])
            nc.sync.dma_start(out=ident[:], in_=id_in[:, :])

            xt = ins.tile([128, KC, N], BF16)
            wq = ins.tile([128, KC, 3 * C], BF16)
            wp = ins.tile([128, KC, C], BF16)

            # HAM warmup: keep TensorE busy during the input DMA wait so the
            # clock gate opens (cold matmuls run at 1.2GHz, warm at 2.4GHz)
            warm_ps = avps.tile([128, N], F32, tag="av")
            warm_rhs = bass.AP(
                tensor=bd[:].tensor, offset=bd[:].offset,
                ap=[list(bd[:].ap[0]), [0, 4], [1, 128]])  # [128, 4, 128] step-0
            for _ in range(2):
                nc.tensor.matmul(warm_ps[0:64, 0:512], bd[:, 0:64], warm_rhs,
                                 start=True, stop=True)

            # keep-warm pads: real 512-row matmuls into a scratch wps tile
            # (no readers; the WAR against later real tiles is free since
            # the PE dispatches in order). When the wps pool is claimed by
            # the pair-2 tail accumulators, pads are dropped (real AV work
            # keeps the PE busy there).
            wps_free = [True]

            def emit_pad():
                if False and wps_free[0]:
                    padt = wps.tile([128, 512], F32, tag="w", name="pad")
                    nc.tensor.matmul(padt[:], bd[:], warm_rhs,
                                     start=True, stop=True)

            xr = xT.rearrange("(k p) n -> p k n", p=128)
            wr = qkv_wT.rearrange("(k p) m -> p k m", p=128)
            for k in range(KC):
                nc.scalar.dma_start(out=xt[:, k, :], in_=xr[:, k, :])
            for k in range(KC):
                nc.sync.dma_start(out=wq[:, k, 0:128], in_=wr[:, k, 0:128])
                nc.sync.dma_start(out=wq[:, k, 384:512], in_=wr[:, k, 384:512])
            for k in range(KC):
                nc.sync.dma_start(out=wq[:, k, 128:256], in_=wr[:, k, 128:256])
                nc.sync.dma_start(out=wq[:, k, 512:640], in_=wr[:, k, 512:640])
                nc.sync.dma_start(out=wq[:, k, 768:1152], in_=wr[:, k, 768:1152])
            for k in range(KC):
                nc.sync.dma_start(out=wq[:, k, 256:384], in_=wr[:, k, 256:384])
                nc.sync.dma_start(out=wq[:, k, 640:768], in_=wr[:, k, 640:768])
            nc.sync.dma_start(
                out=wp[:], in_=proj_wT.rearrange("(k p) m -> p k m", p=128))

            vo = persist.tile([128, TC, H, 128], BF16)
            nc.vector.memset(vo[:, :, :, D:128], 1.0)

            qn = persist.tile([128, 2 * H, N], BF16)    # normalized q|k
            aoT = persist.tile([128, KC, N], BF16)      # attn out (ch, tok)
            pstage = persist.tile([128, KC, N], BF16)   # proj k=0,1 (+bias)

            # ---- emitters ----

            def qkv_chunk_gen(j):
                """QKV output chunk j (128 cols of q|k) + normalization over
                the head dim (partition axis) via block-diag 1/64 matmuls;
                rstd = exp(-0.5*ln(ssq) + ddof bias). 5 PE pieces; both qkr
                copies precede the mean/ssq allocs so no wps tile is ever
                reallocated while an ACT-queued reader still owns it."""
                # Every piece emits the readers of any wps tile it allocates
                # before yielding, so interleaved pads / other lanes can
                # safely reuse pool buffers (the framework can only order a
                # reallocation against readers that already exist).
                qk, qkr, qc, qc2 = {}, {}, {}, {}
                for h in (0, 1):
                    qk[h] = wps.tile([128, 512], F32, tag="w",
                                     name=f"qk{j}_{h}")
                    for k in range(KC):
                        nc.tensor.matmul(
                            qk[h][:],
                            wq[:, k, j * 128:(j + 1) * 128],
                            xt[:, k, h * 512:(h + 1) * 512],
                            start=(k == 0), stop=(k == KC - 1))
                    qkr[h] = wk.tile([128, 512], BF16, tag="qkr",
                                     name=f"qkr{j}_{h}")
                    nc.vector.tensor_copy(qkr[h][:], qk[h][:])
                    yield
                for h in (0, 1):
                    mean = wps.tile([128, 512], F32, tag="w",
                                    name=f"mean{j}_{h}")
                    nc.tensor.matmul(mean[:], bd[:], qkr[h][:],
                                     start=True, stop=True)
                    qc[h] = wk.tile([128, 512], BF16, tag="qc",
                                    name=f"qc{j}_{h}")
                    nc.vector.tensor_sub(qc[h][:], qkr[h][:], mean[:])
                    qc2[h] = wk.tile([128, 512], BF16, tag="qc2",
                                     name=f"qc2{j}_{h}")
                    nc.vector.tensor_mul(qc2[h][:], qc[h][:], qc[h][:])
                yield
                # single [128,1024] SBUF lnv so one merged rstd exp suffices
                lnv = wk.tile([128, 1024], F32, tag="lnv", name=f"lnv{j}")
                for h in (0, 1):
                    ssq = wps.tile([128, 512], F32, tag="w",
                                   name=f"ssq{j}_{h}")
                    nc.tensor.matmul(ssq[:], bd[:], qc2[h][:],
                                     start=True, stop=True)
                    nc.scalar.activation(lnv[:, h * 512:(h + 1) * 512],
                                         ssq[:], LN)
                yield
                rstd = wk.tile([128, 1024], BF16, tag="rstd", name=f"rstd{j}")
                nc.scalar.activation(rstd[:], lnv[:], EXP, scale=-0.5,
                                     bias=ddof_b[:])
                for h in (0, 1):
                    sl = slice(h * 512, (h + 1) * 512)
                    nc.vector.tensor_mul(qn[:, j, sl], qc[h][:], rstd[:, sl])
                emit_pad()
                yield

            def v_gen():
                """V token-major + ones cols already in vo. 8 PE pieces."""
                for t in range(TC):
                    vp = wps.tile([128, 512], F32, tag="w", name=f"vp{t}")
                    for k in range(KC):
                        nc.tensor.matmul(
                            vp[:, 0:C],
                            xt[:, k, t * 128:(t + 1) * 128],
                            wq[:, k, 2 * C:3 * C],
                            start=(k == 0), stop=(k == KC - 1))
                    nc.vector.tensor_copy(
                        vo[:, t, :, 0:D],
                        vp[:, 0:C].rearrange("p (h d) -> p h d", h=H))
                    yield

            def av_head_gen(p, es_list, g):
                """AV for head 2p+g on the avps tile: accumulate over the 8
                k-token chunks; rows 64-127 collect the softmax denominator
                broadcast over 64 partitions by the ones-cols. 4 pieces,
                then 1/denom = exp(-ln d) and the normalize mul."""
                av = avps.tile([128, N], F32, tag="av")
                for tp in range(4):
                    for t in (2 * tp, 2 * tp + 1):
                        es = es_list[2 * t + g]
                        for h5 in (0, 1):
                            nc.tensor.matmul(
                                av[:, h5 * 512:(h5 + 1) * 512],
                                vo[:, t, 2 * p + g, :],
                                es[:, h5 * 512:(h5 + 1) * 512],
                                start=(t == 0), stop=(t == TC - 1))
                    yield
                lnd = recp.tile([64, N], F32, tag="lnd")
                nc.scalar.activation(lnd[:], av[64:128, :], LN)
                rec = recp.tile([64, N], F32, tag="rec")
                nc.scalar.activation(rec[:], lnd[:], EXP, scale=-1.0)
                nc.vector.tensor_mul(aoT[64 * g:64 * (g + 1), p, :],
                                     av[0:64, :], rec[:])

            def av_gen(p, es_list):
                yield from av_head_gen(p, es_list, 0)
                yield from av_head_gen(p, es_list, 1)

            tail = {}

            def av_tail_avps_gen(p, es_list, g):
                """Pair-2 head on the avps tile, self-pacing behind the es
                tiles pair_emit is still producing. Denominator handled in
                the explicit tail."""
                av = None
                for tp in range(4):
                    while len(es_list) < 4 * tp + 4:
                        emit_pad()
                        yield
                    if av is None:
                        av = avps.tile([128, N], F32, tag="av")
                        tail["av"] = av
                    for t in (2 * tp, 2 * tp + 1):
                        es = es_list[2 * t + g]
                        for h5 in (0, 1):
                            nc.tensor.matmul(
                                av[:, h5 * 512:(h5 + 1) * 512],
                                vo[:, t, 2 * p + g, :],
                                es[:, h5 * 512:(h5 + 1) * 512],
                                start=(t == 0), stop=(t == TC - 1))
                    yield

            def av_tail_wps_gen(p, es_list, g):
                """Pair-2 second head accumulated in two wps half-tiles so
                it runs concurrently with the avps head instead of after."""
                avh = None
                for tp in range(4):
                    while len(es_list) < 4 * tp + 4:
                        emit_pad()
                        yield
                    if avh is None:
                        wps_free[0] = False
                        avh = [wps.tile([128, 512], F32, tag="w",
                                        name=f"avt{h5}") for h5 in (0, 1)]
                        tail["avh"] = avh
                    for t in (2 * tp, 2 * tp + 1):
                        es = es_list[2 * t + g]
                        for h5 in (0, 1):
                            nc.tensor.matmul(
                                avh[h5][:],
                                vo[:, t, 2 * p + g, :],
                                es[:, h5 * 512:(h5 + 1) * 512],
                                start=(t == 0), stop=(t == TC - 1))
                    yield

            def proj01_gen():
                """proj partial sums over k=0,1 (pairs 0,1 outputs), bias
                folded into the bf16 staging add. 6 PE pieces."""
                for co in range(KC):
                    for h in (0, 1):
                        pf = wps.tile([128, 512], F32, tag="w",
                                      name=f"p01_{co}_{h}")
                        for k in (0, 1):
                            nc.tensor.matmul(
                                pf[:],
                                wp[:, k, co * 128:(co + 1) * 128],
                                aoT[:, k, h * 512:(h + 1) * 512],
                                start=(k == 0), stop=(k == 1))
                        nc.vector.tensor_scalar_add(
                            pstage[:, co, h * 512:(h + 1) * 512],
                            pf[:], pbt[:, co:co + 1])
                        yield

            def pair_emit(p, filler, es_out, drains=None):
                """Scores + exp for pair p. Per t-chunk the two heads run on
                PE row groups 0/64 (h5-interleaved so they stream
                concurrently); the g-stagger ping-pongs the two s tiles so
                exp(t,g0) overlaps QK(t,g1). 3 filler pieces per t-slot."""
                for t in range(TC):
                    s = {0: sps.tile([128, N], F32, tag="s", name="s0"),
                         1: sps.tile([128, N], F32, tag="s", name="s1")}

                    def qk_mm(g, h5):
                        nc.tensor.matmul(
                            s[g][:, h5 * 512:(h5 + 1) * 512],
                            qn[64 * g:64 * (g + 1), 3 + p,
                               t * 128:(t + 1) * 128],
                            qn[64 * g:64 * (g + 1), p,
                               h5 * 512:(h5 + 1) * 512],
                            start=True, stop=True)

                    qk_mm(0, 0)
                    qk_mm(1, 0)
                    qk_mm(0, 1)
                    es0_t = esp.tile([128, N], BF16, tag="es", name="esg0")
                    nc.scalar.activation(es0_t[:], s[0][:], EXP, scale=scale)
                    qk_mm(1, 1)
                    es1_t = esp.tile([128, N], BF16, tag="es", name="esg1")
                    nc.scalar.activation(es1_t[:], s[1][:], EXP, scale=scale)
                    es_out.append(es0_t)
                    es_out.append(es1_t)
                    for _ in range((drains or [3] * TC)[t]):
                        if next(filler, StopIteration) is StopIteration:
                            emit_pad()

            def chain(*gens):
                for gen in gens:
                    yield from gen

            def spaced1(gen):
                for _ in gen:
                    yield
                    yield

            def interleave(*gens):
                gens = [iter(gg) for gg in gens]
                alive = True
                while alive:
                    alive = False
                    for gg in gens:
                        if next(gg, StopIteration) is not StopIteration:
                            alive = True
                            yield

            def roundrobin_pad(*lanes):
                """Visit lanes cyclically, one piece per visit; an exhausted
                lane contributes a keep-warm pad while any lane remains."""
                lanes = [iter(x) for x in lanes]
                alive = set(range(len(lanes)))
                while alive:
                    for idx in range(len(lanes)):
                        if idx in alive:
                            if next(lanes[idx], StopIteration) is StopIteration:
                                alive.discard(idx)
                                if not alive:
                                    return
                                emit_pad()
                        else:
                            emit_pad()
                        yield

            # ---- schedule ----
            for _ in qkv_chunk_gen(0):
                emit_pad()
            for _ in qkv_chunk_gen(3):
                emit_pad()

            es0, es1, es2 = [], [], []
            pair_emit(0, roundrobin_pad(
                chain(qkv_chunk_gen(1), qkv_chunk_gen(4)), v_gen()), es0)
            pair_emit(1, roundrobin_pad(
                av_gen(0, es0),
                chain(qkv_chunk_gen(2), qkv_chunk_gen(5))), es1)
            # phase E must stay sequential: proj01 reads aoT[:,1,:] which
            # av_gen(1) writes, and both tail accumulators reuse buffers
            # whose previous readers av_gen(1)/proj01 emit.
            filler_e = chain(av_gen(1, es1), proj01_gen(),
                             interleave(av_tail_avps_gen(2, es2, 0),
                                        av_tail_wps_gen(2, es2, 1)))
            pair_emit(2, filler_e, es2, drains=[2, 2, 3, 3, 3, 3, 3, 3])
            for _ in filler_e:
                pass

            # ---- explicit tail: pair-2 normalizers + proj k=2 ----
            av, avh = tail["av"], tail["avh"]
            # head 4 (g0, avps): full-width denominator chain first
            lnd0 = recp.tile([64, N], F32, tag="lnd")
            nc.scalar.activation(lnd0[:], av[64:128, :], LN)
            rec0 = recp.tile([64, N], F32, tag="rec")
            nc.scalar.activation(rec0[:], lnd0[:], EXP, scale=-1.0)
            nc.vector.tensor_mul(aoT[0:64, 2, :], av[0:64, :], rec0[:])
            # head 5 (g1, wps halves) on ACT while g0's mul + proj run
            recs = []
            for h5 in (0, 1):
                lndh = recp.tile([64, 512], F32, tag="lnd2", name=f"lndh{h5}")
                nc.scalar.activation(lndh[:], avh[h5][64:128, :], LN)
                rech = recp.tile([64, 512], F32, tag="rec2", name=f"rech{h5}")
                nc.scalar.activation(rech[:], lndh[:], EXP, scale=-1.0)
                recs.append(rech)
            # proj k=2: co=0's head-4 rows start now (64-row contraction,
            # overlapping head 5's ACT chain); co=1,2 run full-width after.
            pf0 = sps.tile([128, N], F32, tag="s", name="pf0")
            for h5 in (0, 1):
                nc.tensor.matmul(
                    pf0[:, h5 * 512:(h5 + 1) * 512],
                    wp[0:64, 2, 0:128],
                    aoT[0:64, 2, h5 * 512:(h5 + 1) * 512],
                    start=True, stop=False)
            for h5 in (0, 1):
                sl = slice(h5 * 512, (h5 + 1) * 512)
                nc.vector.tensor_mul(aoT[64:128, 2, sl],
                                     avh[h5][0:64, :], recs[h5][:])
            for h5 in (0, 1):
                sl = slice(h5 * 512, (h5 + 1) * 512)
                nc.tensor.matmul(
                    pf0[:, sl], wp[64:128, 2, 0:128], aoT[64:128, 2, sl],
                    start=False, stop=False)
                nc.tensor.matmul(
                    pf0[:, sl], ident[:], pstage[:, 0, sl],
                    start=False, stop=True)
            po0 = pop.tile([128, N], F32, tag="po", name="po0")
            for h5 in (0, 1):
                sl = slice(h5 * 512, (h5 + 1) * 512)
                if h5 == 0:
                    nc.scalar.copy(po0[:, sl], pf0[:, sl])
                else:
                    nc.vector.tensor_copy(po0[:, sl], pf0[:, sl])
                nc.sync.dma_start(out=outT[0:128, sl], in_=po0[:, sl])
            for co in (1, 2):
                pf = sps.tile([128, N], F32, tag="s", name=f"pf{co}")
                for h5 in (0, 1):
                    sl = slice(h5 * 512, (h5 + 1) * 512)
                    nc.tensor.matmul(
                        pf[:, sl],
                        wp[:, 2, co * 128:(co + 1) * 128],
                        aoT[:, 2, sl],
                        start=True, stop=False)
                    # fold the staged k=0,1 partial (+bias) back in
                    nc.tensor.matmul(
                        pf[:, sl], ident[:], pstage[:, co, sl],
                        start=False, stop=True)
                po = pop.tile([128, N], F32, tag="po", name=f"po{co}")
                for h5 in (0, 1):
                    sl = slice(h5 * 512, (h5 + 1) * 512)
                    if (co + h5) % 2 == 0:
                        nc.scalar.copy(po[:, sl], pf[:, sl])
                    else:
                        nc.vector.tensor_copy(po[:, sl], pf[:, sl])
                    nc.sync.dma_start(out=outT[co * 128:(co + 1) * 128, sl],
                                      in_=po[:, sl])

    return nc


def _get_prog():
    global _prog
    if _prog is None:
        _prog = _build()
    return _prog


def _make_in_maps(x, qkv_w, proj_w, proj_b):
    import ml_dtypes

    bf16 = ml_dtypes.bfloat16
    qkv_wT = np.ascontiguousarray(np.asarray(qkv_w, np.float32).T.astype(bf16))
    proj_wT = np.ascontiguousarray(np.asarray(proj_w, np.float32).T.astype(bf16))
    pb = np.ascontiguousarray(
        np.asarray(proj_b, np.float32).reshape(3, 128).T)
    bd = np.zeros((128, 128), np.float32)
    for b0 in (0, 64):
        bd[b0:b0 + 64, b0:b0 + 64] = 1.0 / D   # ddof fix in rstd exp bias
    bd = bd.astype(bf16)
    ident = np.eye(128, dtype=np.float32).astype(bf16)

    shared = {"qkv_wT": qkv_wT, "proj_wT": proj_wT, "pb": pb, "bd": bd,
              "ident": ident}
    x = np.asarray(x, np.float32)
    return [
        {"xT": np.ascontiguousarray(x[b].T.astype(bf16)), **shared}
        for b in range(B)
    ]


def run(x, qkv_w, proj_w, proj_b, trace=False):
    from concourse.bass_utils import run_bass_kernel_spmd

    nc = _get_prog()
    in_maps = _make_in_maps(x, qkv_w, proj_w, proj_b)
    res = run_bass_kernel_spmd(
        nc, in_maps, core_ids=list(range(NCORES)), trace=trace)
    out = np.stack([res.results[b]["outT"].T for b in range(B)])
    return np.ascontiguousarray(out.astype(np.float32)), res


def kernel(x, qkv_w, proj_w, proj_b):
    out, _ = run(x, qkv_w, proj_w, proj_b)
    return out


# revision 37
# speedup vs baseline: 1.0166x; 1.0166x over previous
"""Trainium2 Bass kernel for nn_Attention_41085657153633.

B=8, N=1024, C=384, H=6, D=64 attention with per-head q/k normalization
(mean/std over head_dim, ddof=1), softmax(QK^T/sqrt(D)) @ V, output proj.

Sharding: data-parallel over B — each of the 8 NeuronCores handles one
batch element end-to-end (no collectives).

v4 design (ACT-bound pipeline, ~all-bf16):
  - all matmul operands bf16 (FWL weight loads, half DMA, 2x DVE modes).
  - scores per (t-chunk, head): the two heads of a pair run CONCURRENTLY
    on PE row groups 0/64; the g-stagger ping-pongs the two s tiles so
    exp(t,g0) overlaps QK(t,g1) and ACT (the bottleneck engine) stays fed.
  - softmax denominator via 64 ones-columns in the AV weights (free in PE
    streaming); 1/denom = exp(-ln d) on ACT (ln/exp share one table set).
  - PE filler work (next chunks' QKV+norm, V, previous pair's AV, proj
    partials) is spread round-robin between QK t-steps; keep-warm pad
    matmuls fill any slack so the HAM activity monitor never re-throttles
    the PE clock to 1.2GHz mid-kernel.
  - norm chain allocs both PSUM mean/ssq tiles after both qkr copies so
    no matmul ever waits an ACT-queued reader (wps pool is 2-deep).
  - proj: k=0,1 partials (+bias) staged to SBUF bf16 during pair 2; at
    the tail k=2 is split into two 64-row contraction halves (head 4
    right after its normalizer, head 5 after the last denominator), the
    staged partial is folded back via an identity matmul, and the
    PSUM->SBUF moves alternate ACT/DVE so the three output chunks drain
    in parallel with the DMA.
"""

import sys

sys.path.insert(0, "/opt/trn_rl_repo")

import json

import numpy as np

B, N, C = 8, 1024, 384
H, D = 6, 64
NCORES = 8
KC = C // 128   # 3 contraction chunks of the model dim
TC = N // 128   # 8 token chunks

_prog = None


def _install_multiwait_fixup():
    """This container's walrus build rejects >1 sync wait per instruction
    ("Too many sync wait commands"). Rewrite the BIR JSON before compile:
    hoist extra waits onto single-wait EventSemaphore instructions
    inserted just before the owner on the same engine (engines dispatch
    in program order, so the gating is preserved)."""
    from concourse import bass2jax, bass_utils

    if getattr(bass_utils, "_multiwait_fixup", False):
        return
    bass_utils._multiwait_fixup = True

    orig = bass_utils.compile_bir_kernel

    def _split(bir_json: bytes) -> bytes:
        j = json.loads(bir_json)
        for fn in j.get("functions", []):
            for bb in fn.get("blocks", []):
                out = []
                for inst in bb.get("instructions", []):
                    si = inst.get("sync_info")
                    waits = si.get("on_wait", []) if si else []
                    if len(waits) > 1:
                        for k, w in enumerate(waits[:-1]):
                            out.append({
                                "debug": inst.get("debug", 0),
                                "engine": inst["engine"],
                                "ins": [],
                                "outs": [],
                                "name": f"{inst['name']}-sw{k}",
                                "opcode": "EventSemaphore",
                                "sync_info": {"on_update": [], "on_wait": [w]},
                            })
                        si["on_wait"] = [waits[-1]]
                    out.append(inst)
                bb["instructions"] = out
        return json.dumps(j).encode()

    def patched(bir_json, tmpdir, neff_name="file.neff"):
        return orig(_split(bir_json), tmpdir, neff_name)

    bass_utils.compile_bir_kernel = patched
    bass2jax.compile_bir_kernel = patched


def _build():
    import concourse.bass as bass
    import concourse.tile as tile
    from concourse import mybir

    _install_multiwait_fixup()

    F32 = mybir.dt.float32
    BF16 = mybir.dt.bfloat16
    EXP = mybir.ActivationFunctionType.Exp
    LN = mybir.ActivationFunctionType.Ln

    nc = bass.Bass("TRN2")
    xT = nc.dram_tensor("xT", [C, N], BF16, kind="ExternalInput")
    qkv_wT = nc.dram_tensor("qkv_wT", [C, 3 * C], BF16, kind="ExternalInput")
    proj_wT = nc.dram_tensor("proj_wT", [C, C], BF16, kind="ExternalInput")
    pb = nc.dram_tensor("pb", [128, 3], F32, kind="ExternalInput")
    bd_in = nc.dram_tensor("bd", [128, 128], BF16, kind="ExternalInput")
    id_in = nc.dram_tensor("ident", [128, 128], BF16, kind="ExternalInput")
    outT = nc.dram_tensor("outT", [C, N], F32, kind="ExternalOutput")

    scale = float(D) ** -0.5

    with tile.TileContext(nc) as tc:
      with nc.allow_low_precision(reason="bf16 matmul/elementwise pipeline"):
        with tc.tile_pool(name="consts", bufs=1) as consts, \
             tc.tile_pool(name="ins", bufs=1) as ins, \
             tc.tile_pool(name="persist", bufs=1) as persist, \
             tc.tile_pool(name="wk", bufs=3) as wk, \
             tc.tile_pool(name="es", bufs=32) as esp, \
             tc.tile_pool(name="rec", bufs=3) as recp, \
             tc.tile_pool(name="po", bufs=2) as pop, \
             tc.tile_pool(name="sps", bufs=2, space="PSUM") as sps, \
             tc.tile_pool(name="avps", bufs=1, space="PSUM") as avps, \
             tc.tile_pool(name="wps", bufs=2, space="PSUM") as wps:

            # ---- consts + ACT table preload ----
            bd = consts.tile([128, 128], BF16)
            ident = consts.tile([128, 128], BF16)
            pbt = consts.tile([128, 3], F32)
            ddof_b = consts.tile([128, 1], F32)
            nc.vector.memset(ddof_b[:], -0.5 * float(np.log(64.0 / 63.0)))
            dum = consts.tile([1, 16], F32)
            nc.vector.memset(dum[:], 1.0)
            dum2 = consts.tile([1, 16], F32)
            # force the natural_log_exp table set resident during DMA wait
            nc.scalar.activation(dum2[:], dum[:], LN)
            nc.scalar.activation(dum2[:], dum[:], EXP)

            nc.sync.dma_start(out=bd[:], in_=bd_in[:, :])
            nc.sync.dma_start(out=pbt[:], in_=pb[:, :])
            nc.sync.dma_start(out=ident[:], in_=id_in[:, :])

            xt = ins.tile([128, KC, N], BF16)
            wq = ins.tile([128, KC, 3 * C], BF16)
            wp = ins.tile([128, KC, C], BF16)

            # HAM warmup: keep TensorE busy during the input DMA wait so the
            # clock gate opens (cold matmuls run at 1.2GHz, warm at 2.4GHz)
            warm_ps = avps.tile([128, N], F32, tag="av")
            warm_rhs = bass.AP(
                tensor=bd[:].tensor, offset=bd[:].offset,
                ap=[list(bd[:].ap[0]), [0, 4], [1, 128]])  # [128, 4, 128] step-0
            for _ in range(2):
                nc.tensor.matmul(warm_ps[0:64, 0:512], bd[:, 0:64], warm_rhs,
                                 start=True, stop=True)

            # keep-warm pads: real 512-row matmuls into a scratch wps tile
            # (no readers; the WAR against later real tiles is free since
            # the PE dispatches in order). When the wps pool is claimed by
            # the pair-2 tail accumulators, pads are dropped (real AV work
            # keeps the PE busy there).
            wps_free = [True]

            def emit_pad():
                if False and wps_free[0]:
                    padt = wps.tile([128, 512], F32, tag="w", name="pad")
                    nc.tensor.matmul(padt[:], bd[:], warm_rhs,
                                     start=True, stop=True)

            xr = xT.rearrange("(k p) n -> p k n", p=128)
            wr = qkv_wT.rearrange("(k p) m -> p k m", p=128)
            for k in range(KC):
                nc.scalar.dma_start(out=xt[:, k, :], in_=xr[:, k, :])
            for k in range(KC):
                nc.sync.dma_start(out=wq[:, k, 0:128], in_=wr[:, k, 0:128])
                nc.sync.dma_start(out=wq[:, k, 384:512], in_=wr[:, k, 384:512])
            for k in range(KC):
                nc.sync.dma_start(out=wq[:, k, 128:256], in_=wr[:, k, 128:256])
                nc.sync.dma_start(out=wq[:, k, 512:640], in_=wr[:, k, 512:640])
                nc.sync.dma_start(out=wq[:, k, 768:1152], in_=wr[:, k, 768:1152])
            for k in range(KC):
                nc.sync.dma_start(out=wq[:, k, 256:384], in_=wr[:, k, 256:384])
                nc.sync.dma_start(out=wq[:, k, 640:768], in_=wr[:, k, 640:768])
            nc.sync.dma_start(
                out=wp[:], in_=proj_wT.rearrange("(k p) m -> p k m", p=128))

            vo = persist.tile([128, TC, H, 128], BF16)
            nc.vector.memset(vo[:, :, :, D:128], 1.0)

            qn = persist.tile([128, 2 * H, N], BF16)    # normalized q|k
            aoT = persist.tile([128, KC, N], BF16)      # attn out (ch, tok)
            pstage = persist.tile([128, KC, N], BF16)   # proj k=0,1 (+bias)

            # ---- emitters ----

            def qkv_chunk_gen(j):
                """QKV output chunk j (128 cols of q|k) + normalization over
                the head dim (partition axis) via block-diag 1/64 matmuls;
                rstd = exp(-0.5*ln(ssq) + ddof bias). 5 PE pieces; both qkr
                copies precede the mean/ssq allocs so no wps tile is ever
                reallocated while an ACT-queued reader still owns it."""
                # Every piece emits the readers of any wps tile it allocates
                # before yielding, so interleaved pads / other lanes can
                # safely reuse pool buffers (the framework can only order a
                # reallocation against readers that already exist).
                qk, qkr, qc, qc2 = {}, {}, {}, {}
                for h in (0, 1):
                    qk[h] = wps.tile([128, 512], F32, tag="w",
                                     name=f"qk{j}_{h}")
                    for k in range(KC):
                        nc.tensor.matmul(
                            qk[h][:],
                            wq[:, k, j * 128:(j + 1) * 128],
                            xt[:, k, h * 512:(h + 1) * 512],
                            start=(k == 0), stop=(k == KC - 1))
                    qkr[h] = wk.tile([128, 512], BF16, tag="qkr",
                                     name=f"qkr{j}_{h}")
                    nc.vector.tensor_copy(qkr[h][:], qk[h][:])
                    yield
                for h in (0, 1):
                    mean = wps.tile([128, 512], F32, tag="w",
                                    name=f"mean{j}_{h}")
                    nc.tensor.matmul(mean[:], bd[:], qkr[h][:],
                                     start=True, stop=True)
                    qc[h] = wk.tile([128, 512], BF16, tag="qc",
                                    name=f"qc{j}_{h}")
                    nc.vector.tensor_sub(qc[h][:], qkr[h][:], mean[:])
                    qc2[h] = wk.tile([128, 512], BF16, tag="qc2",
                                     name=f"qc2{j}_{h}")
                    nc.vector.tensor_mul(qc2[h][:], qc[h][:], qc[h][:])
                yield
                # single [128,1024] SBUF lnv so one merged rstd exp suffices
                lnv = wk.tile([128, 1024], F32, tag="lnv", name=f"lnv{j}")
                for h in (0, 1):
                    ssq = wps.tile([128, 512], F32, tag="w",
                                   name=f"ssq{j}_{h}")
                    nc.tensor.matmul(ssq[:], bd[:], qc2[h][:],
                                     start=True, stop=True)
                    nc.scalar.activation(lnv[:, h * 512:(h + 1) * 512],
                                         ssq[:], LN)
                yield
                rstd = wk.tile([128, 1024], BF16, tag="rstd", name=f"rstd{j}")
                nc.scalar.activation(rstd[:], lnv[:], EXP, scale=-0.5,
                                     bias=ddof_b[:])
                for h in (0, 1):
                    sl = slice(h * 512, (h + 1) * 512)
                    nc.vector.tensor_mul(qn[:, j, sl], qc[h][:], rstd[:, sl])
                emit_pad()
                yield

            def v_gen():
                """V token-major + ones cols already in vo. 8 PE pieces."""
                for t in range(TC):
                    vp = wps.tile([128, 512], F32, tag="w", name=f"vp{t}")
                    for k in range(KC):
                        nc.tensor.matmul(
                            vp[:, 0:C],
                            xt[:, k, t * 128:(t + 1) * 128],
                            wq[:, k, 2 * C:3 * C],
                            start=(k == 0), stop=(k == KC - 1))
                    nc.vector.tensor_copy(
                        vo[:, t, :, 0:D],
                        vp[:, 0:C].rearrange("p (h d) -> p h d", h=H))
                    yield

            def av_head_gen(p, es_list, g):
                """AV for head 2p+g on the avps tile: accumulate over the 8
                k-token chunks; rows 64-127 collect the softmax denominator
                broadcast over 64 partitions by the ones-cols. 4 pieces,
                then 1/denom = exp(-ln d) and the normalize mul."""
                av = avps.tile([128, N], F32, tag="av")
                for tp in range(4):
                    for t in (2 * tp, 2 * tp + 1):
                        es = es_list[2 * t + g]
                        for h5 in (0, 1):
                            nc.tensor.matmul(
                                av[:, h5 * 512:(h5 + 1) * 512],
                                vo[:, t, 2 * p + g, :],
                                es[:, h5 * 512:(h5 + 1) * 512],
                                start=(t == 0), stop=(t == TC - 1))
                    yield
                lnd = recp.tile([64, N], F32, tag="lnd")
                nc.scalar.activation(lnd[:], av[64:128, :], LN)
                rec = recp.tile([64, N], F32, tag="rec")
                nc.scalar.activation(rec[:], lnd[:], EXP, scale=-1.0)
                nc.vector.tensor_mul(aoT[64 * g:64 * (g + 1), p, :],
                                     av[0:64, :], rec[:])

            def av_gen(p, es_list):
                yield from av_head_gen(p, es_list, 0)
                yield from av_head_gen(p, es_list, 1)

            tail = {}

            def av_tail_avps_gen(p, es_list, g):
                """Pair-2 head on the avps tile, self-pacing behind the es
                tiles pair_emit is still producing. Denominator handled in
                the explicit tail."""
                av = None
                for tp in range(4):
                    while len(es_list) < 4 * tp + 4:
                        emit_pad()
                        yield
                    if av is None:
                        av = avps.tile([128, N], F32, tag="av")
                        tail["av"] = av
                    for t in (2 * tp, 2 * tp + 1):
                        es = es_list[2 * t + g]
                        for h5 in (0, 1):
                            nc.tensor.matmul(
                                av[:, h5 * 512:(h5 + 1) * 512],
                                vo[:, t, 2 * p + g, :],
                                es[:, h5 * 512:(h5 + 1) * 512],
                                start=(t == 0), stop=(t == TC - 1))
                    yield

            def av_tail_wps_gen(p, es_list, g):
                """Pair-2 second head accumulated in two wps half-tiles so
                it runs concurrently with the avps head instead of after."""
                avh = None
                for tp in range(4):
                    while len(es_list) < 4 * tp + 4:
                        emit_pad()
                        yield
                    if avh is None:
                        wps_free[0] = False
                        avh = [wps.tile([128, 512], F32, tag="w",
                                        name=f"avt{h5}") for h5 in (0, 1)]
                        tail["avh"] = avh
                    for t in (2 * tp, 2 * tp + 1):
                        es = es_list[2 * t + g]
                        for h5 in (0, 1):
                            nc.tensor.matmul(
                                avh[h5][:],
                                vo[:, t, 2 * p + g, :],
                                es[:, h5 * 512:(h5 + 1) * 512],
                                start=(t == 0), stop=(t == TC - 1))
                    yield

            def proj01_gen():
                """proj partial sums over k=0,1 (pairs 0,1 outputs), bias
                folded into the bf16 staging add. 6 PE pieces."""
                for co in range(KC):
                    for h in (0, 1):
                        pf = wps.tile([128, 512], F32, tag="w",
                                      name=f"p01_{co}_{h}")
                        for k in (0, 1):
                            nc.tensor.matmul(
                                pf[:],
                                wp[:, k, co * 128:(co + 1) * 128],
                                aoT[:, k, h * 512:(h + 1) * 512],
                                start=(k == 0), stop=(k == 1))
                        nc.vector.tensor_scalar_add(
                            pstage[:, co, h * 512:(h + 1) * 512],
                            pf[:], pbt[:, co:co + 1])
                        yield

            def pair_emit(p, filler, es_out):
                """Scores + exp for pair p. Per t-chunk the two heads run on
                PE row groups 0/64 (h5-interleaved so they stream
                concurrently); the g-stagger ping-pongs the two s tiles so
                exp(t,g0) overlaps QK(t,g1). 3 filler pieces per t-slot."""
                for t in range(TC):
                    s = {0: sps.tile([128, N], F32, tag="s", name="s0"),
                         1: sps.tile([128, N], F32, tag="s", name="s1")}

                    def qk_mm(g, h5):
                        nc.tensor.matmul(
                            s[g][:, h5 * 512:(h5 + 1) * 512],
                            qn[64 * g:64 * (g + 1), 3 + p,
                               t * 128:(t + 1) * 128],
                            qn[64 * g:64 * (g + 1), p,
                               h5 * 512:(h5 + 1) * 512],
                            start=True, stop=True)

                    qk_mm(0, 0)
                    qk_mm(1, 0)
                    qk_mm(0, 1)
                    es0_t = esp.tile([128, N], BF16, tag="es", name="esg0")
                    nc.scalar.activation(es0_t[:], s[0][:], EXP, scale=scale)
                    qk_mm(1, 1)
                    es1_t = esp.tile([128, N], BF16, tag="es", name="esg1")
                    nc.scalar.activation(es1_t[:], s[1][:], EXP, scale=scale)
                    es_out.append(es0_t)
                    es_out.append(es1_t)
                    for _ in range(3):
                        if next(filler, StopIteration) is StopIteration:
                            emit_pad()

            def chain(*gens):
                for gen in gens:
                    yield from gen

            def spaced1(gen):
                for _ in gen:
                    yield
                    yield

            def interleave(*gens):
                gens = [iter(gg) for gg in gens]
                alive = True
                while alive:
                    alive = False
                    for gg in gens:
                        if next(gg, StopIteration) is not StopIteration:
                            alive = True
                            yield

            def roundrobin_pad(*lanes):
                """Visit lanes cyclically, one piece per visit; an exhausted
                lane contributes a keep-warm pad while any lane remains."""
                lanes = [iter(x) for x in lanes]
                alive = set(range(len(lanes)))
                while alive:
                    for idx in range(len(lanes)):
                        if idx in alive:
                            if next(lanes[idx], StopIteration) is StopIteration:
                                alive.discard(idx)
                                if not alive:
                                    return
                                emit_pad()
                        else:
                            emit_pad()
                        yield

            # ---- schedule ----
            for _ in qkv_chunk_gen(0):
                emit_pad()
            for _ in qkv_chunk_gen(3):
                emit_pad()

            es0, es1, es2 = [], [], []
            pair_emit(0, roundrobin_pad(
                chain(qkv_chunk_gen(1), qkv_chunk_gen(4)), v_gen()), es0)
            pair_emit(1, roundrobin_pad(
                av_gen(0, es0),
                chain(qkv_chunk_gen(2), qkv_chunk_gen(5))), es1)
            # phase E must stay sequential: proj01 reads aoT[:,1,:] which
            # av_gen(1) writes, and both tail accumulators reuse buffers
            # whose previous readers av_gen(1)/proj01 emit.
            filler_e = chain(av_gen(1, es1), proj01_gen(),
                             interleave(av_tail_avps_gen(2, es2, 0),
                                        av_tail_wps_gen(2, es2, 1)))
            pair_emit(2, filler_e, es2)
            for _ in filler_e:
                pass

            # ---- explicit tail: pair-2 normalizers + proj k=2 ----
            av, avh = tail["av"], tail["avh"]
            # head 4 (g0, avps): full-width denominator chain first
            lnd0 = recp.tile([64, N], F32, tag="lnd")
            nc.scalar.activation(lnd0[:], av[64:128, :], LN)
            rec0 = recp.tile([64, N], F32, tag="rec")
            nc.scalar.activation(rec0[:], lnd0[:], EXP, scale=-1.0)
            nc.vector.tensor_mul(aoT[0:64, 2, :], av[0:64, :], rec0[:])
            # head 5 (g1, wps halves) on ACT while g0's mul + proj run
            recs = []
            for h5 in (0, 1):
                lndh = recp.tile([64, 512], F32, tag="lnd2", name=f"lndh{h5}")
                nc.scalar.activation(lndh[:], avh[h5][64:128, :], LN)
                rech = recp.tile([64, 512], F32, tag="rec2", name=f"rech{h5}")
                nc.scalar.activation(rech[:], lndh[:], EXP, scale=-1.0)
                recs.append(rech)
            # proj k=2: co=0's head-4 rows start now (64-row contraction,
            # overlapping head 5's ACT chain); co=1,2 run full-width after.
            pf0 = sps.tile([128, N], F32, tag="s", name="pf0")
            for h5 in (0, 1):
                nc.tensor.matmul(
                    pf0[:, h5 * 512:(h5 + 1) * 512],
                    wp[0:64, 2, 0:128],
                    aoT[0:64, 2, h5 * 512:(h5 + 1) * 512],
                    start=True, stop=False)
            for h5 in (0, 1):
                sl = slice(h5 * 512, (h5 + 1) * 512)
                nc.vector.tensor_mul(aoT[64:128, 2, sl],
                                     avh[h5][0:64, :], recs[h5][:])
            for h5 in (0, 1):
                sl = slice(h5 * 512, (h5 + 1) * 512)
                nc.tensor.matmul(
                    pf0[:, sl], wp[64:128, 2, 0:128], aoT[64:128, 2, sl],
                    start=False, stop=False)
                nc.tensor.matmul(
                    pf0[:, sl], ident[:], pstage[:, 0, sl],
                    start=False, stop=True)
            po0 = pop.tile([128, N], F32, tag="po", name="po0")
            for h5 in (0, 1):
                sl = slice(h5 * 512, (h5 + 1) * 512)
                if h5 == 0:
                    nc.scalar.copy(po0[:, sl], pf0[:, sl])
                else:
                    nc.vector.tensor_copy(po0[:, sl], pf0[:, sl])
                nc.sync.dma_start(out=outT[0:128, sl], in_=po0[:, sl])
            for co in (1, 2):
                pf = sps.tile([128, N], F32, tag="s", name=f"pf{co}")
                for h5 in (0, 1):
                    sl = slice(h5 * 512, (h5 + 1) * 512)
                    nc.tensor.matmul(
                        pf[:, sl],
                        wp[:, 2, co * 128:(co + 1) * 128],
                        aoT[:, 2, sl],
                        start=True, stop=False)
                    # fold the staged k=0,1 partial (+bias) back in
                    nc.tensor.matmul(
                        pf[:, sl], ident[:], pstage[:, co, sl],
                        start=False, stop=True)
                po = pop.tile([128, N], F32, tag="po", name=f"po{co}")
                for h5 in (0, 1):
                    sl = slice(h5 * 512, (h5 + 1) * 512)
                    if (co + h5) % 2 == 0:
                        nc.scalar.copy(po[:, sl], pf[:, sl])
                    else:
                        nc.vector.tensor_copy(po[:, sl], pf[:, sl])
                    nc.sync.dma_start(out=outT[co * 128:(co + 1) * 128, sl],
                                      in_=po[:, sl])

    return nc


def _get_prog():
    global _prog
    if _prog is None:
        _prog = _build()
    return _prog


def _make_in_maps(x, qkv_w, proj_w, proj_b):
    import ml_dtypes

    bf16 = ml_dtypes.bfloat16
    qkv_wT = np.ascontiguousarray(np.asarray(qkv_w, np.float32).T.astype(bf16))
    proj_wT = np.ascontiguousarray(np.asarray(proj_w, np.float32).T.astype(bf16))
    pb = np.ascontiguousarray(
        np.asarray(proj_b, np.float32).reshape(3, 128).T)
    bd = np.zeros((128, 128), np.float32)
    for b0 in (0, 64):
        bd[b0:b0 + 64, b0:b0 + 64] = 1.0 / D   # ddof fix in rstd exp bias
    bd = bd.astype(bf16)
    ident = np.eye(128, dtype=np.float32).astype(bf16)

    shared = {"qkv_wT": qkv_wT, "proj_wT": proj_wT, "pb": pb, "bd": bd,
              "ident": ident}
    x = np.asarray(x, np.float32)
    return [
        {"xT": np.ascontiguousarray(x[b].T.astype(bf16)), **shared}
        for b in range(B)
    ]


def run(x, qkv_w, proj_w, proj_b, trace=False):
    from concourse.bass_utils import run_bass_kernel_spmd

    nc = _get_prog()
    in_maps = _make_in_maps(x, qkv_w, proj_w, proj_b)
    res = run_bass_kernel_spmd(
        nc, in_maps, core_ids=list(range(NCORES)), trace=trace)
    out = np.stack([res.results[b]["outT"].T for b in range(B)])
    return np.ascontiguousarray(out.astype(np.float32)), res


def kernel(x, qkv_w, proj_w, proj_b):
    out, _ = run(x, qkv_w, proj_w, proj_b)
    return out


# revision 38
# speedup vs baseline: 1.0308x; 1.0140x over previous
"""Trainium2 Bass kernel for nn_Attention_41085657153633.

B=8, N=1024, C=384, H=6, D=64 attention with per-head q/k normalization
(mean/std over head_dim, ddof=1), softmax(QK^T/sqrt(D)) @ V, output proj.

Sharding: data-parallel over B — each of the 8 NeuronCores handles one
batch element end-to-end (no collectives).

v4 design (ACT-bound pipeline, ~all-bf16):
  - all matmul operands bf16 (FWL weight loads, half DMA, 2x DVE modes).
  - scores per (t-chunk, head): the two heads of a pair run CONCURRENTLY
    on PE row groups 0/64; the g-stagger ping-pongs the two s tiles so
    exp(t,g0) overlaps QK(t,g1) and ACT (the bottleneck engine) stays fed.
  - softmax denominator via 64 ones-columns in the AV weights (free in PE
    streaming); 1/denom = exp(-ln d) on ACT (ln/exp share one table set).
  - PE filler work (next chunks' QKV+norm, V, previous pair's AV, proj
    partials) is spread round-robin between QK t-steps; keep-warm pad
    matmuls fill any slack so the HAM activity monitor never re-throttles
    the PE clock to 1.2GHz mid-kernel.
  - norm chain allocs both PSUM mean/ssq tiles after both qkr copies so
    no matmul ever waits an ACT-queued reader (wps pool is 2-deep).
  - proj: k=0,1 partials (+bias) staged to SBUF bf16 during pair 2; at
    the tail k=2 is split into two 64-row contraction halves (head 4
    right after its normalizer, head 5 after the last denominator), the
    staged partial is folded back via an identity matmul, and the
    PSUM->SBUF moves alternate ACT/DVE so the three output chunks drain
    in parallel with the DMA.
"""

import sys

sys.path.insert(0, "/opt/trn_rl_repo")

import json

import numpy as np

B, N, C = 8, 1024, 384
H, D = 6, 64
NCORES = 8
KC = C // 128   # 3 contraction chunks of the model dim
TC = N // 128   # 8 token chunks

_prog = None


def _install_multiwait_fixup():
    """This container's walrus build rejects >1 sync wait per instruction
    ("Too many sync wait commands"). Rewrite the BIR JSON before compile:
    hoist extra waits onto single-wait EventSemaphore instructions
    inserted just before the owner on the same engine (engines dispatch
    in program order, so the gating is preserved)."""
    from concourse import bass2jax, bass_utils

    if getattr(bass_utils, "_multiwait_fixup", False):
        return
    bass_utils._multiwait_fixup = True

    orig = bass_utils.compile_bir_kernel

    def _split(bir_json: bytes) -> bytes:
        j = json.loads(bir_json)
        for fn in j.get("functions", []):
            for bb in fn.get("blocks", []):
                out = []
                for inst in bb.get("instructions", []):
                    si = inst.get("sync_info")
                    waits = si.get("on_wait", []) if si else []
                    if len(waits) > 1:
                        for k, w in enumerate(waits[:-1]):
                            out.append({
                                "debug": inst.get("debug", 0),
                                "engine": inst["engine"],
                                "ins": [],
                                "outs": [],
                                "name": f"{inst['name']}-sw{k}",
                                "opcode": "EventSemaphore",
                                "sync_info": {"on_update": [], "on_wait": [w]},
                            })
                        si["on_wait"] = [waits[-1]]
                    out.append(inst)
                bb["instructions"] = out
        return json.dumps(j).encode()

    def patched(bir_json, tmpdir, neff_name="file.neff"):
        return orig(_split(bir_json), tmpdir, neff_name)

    bass_utils.compile_bir_kernel = patched
    bass2jax.compile_bir_kernel = patched


def _build():
    import concourse.bass as bass
    import concourse.tile as tile
    from concourse import mybir

    _install_multiwait_fixup()

    F32 = mybir.dt.float32
    BF16 = mybir.dt.bfloat16
    EXP = mybir.ActivationFunctionType.Exp
    LN = mybir.ActivationFunctionType.Ln

    nc = bass.Bass("TRN2")
    xT = nc.dram_tensor("xT", [C, N], BF16, kind="ExternalInput")
    qkv_wT = nc.dram_tensor("qkv_wT", [C, 3 * C], BF16, kind="ExternalInput")
    proj_wT = nc.dram_tensor("proj_wT", [C, C], BF16, kind="ExternalInput")
    pb = nc.dram_tensor("pb", [128, 3], F32, kind="ExternalInput")
    bd_in = nc.dram_tensor("bd", [128, 128], BF16, kind="ExternalInput")
    id_in = nc.dram_tensor("ident", [128, 128], BF16, kind="ExternalInput")
    outT = nc.dram_tensor("outT", [C, N], F32, kind="ExternalOutput")

    scale = float(D) ** -0.5

    with tile.TileContext(nc) as tc:
      with nc.allow_low_precision(reason="bf16 matmul/elementwise pipeline"):
        with tc.tile_pool(name="consts", bufs=1) as consts, \
             tc.tile_pool(name="ins", bufs=1) as ins, \
             tc.tile_pool(name="persist", bufs=1) as persist, \
             tc.tile_pool(name="wk", bufs=3) as wk, \
             tc.tile_pool(name="es", bufs=32) as esp, \
             tc.tile_pool(name="rec", bufs=3) as recp, \
             tc.tile_pool(name="po", bufs=2) as pop, \
             tc.tile_pool(name="sps", bufs=2, space="PSUM") as sps, \
             tc.tile_pool(name="avps", bufs=1, space="PSUM") as avps, \
             tc.tile_pool(name="wps", bufs=2, space="PSUM") as wps:

            # ---- consts + ACT table preload ----
            bd = consts.tile([128, 128], BF16)
            ident = consts.tile([128, 128], BF16)
            pbt = consts.tile([128, 3], F32)
            ddof_b = consts.tile([128, 1], F32)
            nc.vector.memset(ddof_b[:], -0.5 * float(np.log(64.0 / 63.0)))
            dum = consts.tile([1, 16], F32)
            nc.vector.memset(dum[:], 1.0)
            dum2 = consts.tile([1, 16], F32)
            # force the natural_log_exp table set resident during DMA wait
            nc.scalar.activation(dum2[:], dum[:], LN)
            nc.scalar.activation(dum2[:], dum[:], EXP)

            nc.sync.dma_start(out=bd[:], in_=bd_in[:, :])
            nc.sync.dma_start(out=pbt[:], in_=pb[:, :])
            nc.sync.dma_start(out=ident[:], in_=id_in[:, :])

            xt = ins.tile([128, KC, N], BF16)
            wq = ins.tile([128, KC, 3 * C], BF16)
            wp = ins.tile([128, KC, C], BF16)

            # HAM warmup: keep TensorE busy during the input DMA wait so the
            # clock gate opens (cold matmuls run at 1.2GHz, warm at 2.4GHz)
            warm_ps = avps.tile([128, N], F32, tag="av")
            warm_rhs = bass.AP(
                tensor=bd[:].tensor, offset=bd[:].offset,
                ap=[list(bd[:].ap[0]), [0, 4], [1, 128]])  # [128, 4, 128] step-0
            for _ in range(2):
                nc.tensor.matmul(warm_ps[0:64, 0:512], bd[:, 0:64], warm_rhs,
                                 start=True, stop=True)

            # keep-warm pads: real 512-row matmuls into a scratch wps tile
            # (no readers; the WAR against later real tiles is free since
            # the PE dispatches in order). When the wps pool is claimed by
            # the pair-2 tail accumulators, pads are dropped (real AV work
            # keeps the PE busy there).
            wps_free = [True]

            def emit_pad():
                if False and wps_free[0]:
                    padt = wps.tile([128, 512], F32, tag="w", name="pad")
                    nc.tensor.matmul(padt[:], bd[:], warm_rhs,
                                     start=True, stop=True)

            xr = xT.rearrange("(k p) n -> p k n", p=128)
            wr = qkv_wT.rearrange("(k p) m -> p k m", p=128)
            for k in range(KC):
                nc.scalar.dma_start(out=xt[:, k, :], in_=xr[:, k, :])
            for k in range(KC):
                nc.sync.dma_start(out=wq[:, k, 0:128], in_=wr[:, k, 0:128])
                nc.sync.dma_start(out=wq[:, k, 384:512], in_=wr[:, k, 384:512])
            for k in range(KC):
                nc.sync.dma_start(out=wq[:, k, 128:256], in_=wr[:, k, 128:256])
                nc.sync.dma_start(out=wq[:, k, 512:640], in_=wr[:, k, 512:640])
                nc.sync.dma_start(out=wq[:, k, 768:1152], in_=wr[:, k, 768:1152])
            for k in range(KC):
                nc.sync.dma_start(out=wq[:, k, 256:384], in_=wr[:, k, 256:384])
                nc.sync.dma_start(out=wq[:, k, 640:768], in_=wr[:, k, 640:768])
            nc.sync.dma_start(
                out=wp[:], in_=proj_wT.rearrange("(k p) m -> p k m", p=128))

            vo = persist.tile([128, TC, H, 128], BF16)
            nc.vector.memset(vo[:, :, :, D:128], 1.0)

            qn = persist.tile([128, 2 * H, N], BF16)    # normalized q|k
            aoT = persist.tile([128, KC, N], BF16)      # attn out (ch, tok)
            pstage = persist.tile([128, KC, N], BF16)   # proj k=0,1 (+bias)

            # ---- emitters ----

            def qkv_chunk_gen(j):
                """QKV output chunk j (128 cols of q|k) + normalization over
                the head dim (partition axis) via block-diag 1/64 matmuls;
                rstd = exp(-0.5*ln(ssq) + ddof bias). 5 PE pieces; both qkr
                copies precede the mean/ssq allocs so no wps tile is ever
                reallocated while an ACT-queued reader still owns it."""
                # Every piece emits the readers of any wps tile it allocates
                # before yielding, so interleaved pads / other lanes can
                # safely reuse pool buffers (the framework can only order a
                # reallocation against readers that already exist).
                qk, qkr, qc, qc2 = {}, {}, {}, {}
                for h in (0, 1):
                    qk[h] = wps.tile([128, 512], F32, tag="w",
                                     name=f"qk{j}_{h}")
                    for k in range(KC):
                        nc.tensor.matmul(
                            qk[h][:],
                            wq[:, k, j * 128:(j + 1) * 128],
                            xt[:, k, h * 512:(h + 1) * 512],
                            start=(k == 0), stop=(k == KC - 1))
                    qkr[h] = wk.tile([128, 512], BF16, tag="qkr",
                                     name=f"qkr{j}_{h}")
                    nc.vector.tensor_copy(qkr[h][:], qk[h][:])
                    yield
                for h in (0, 1):
                    mean = wps.tile([128, 512], F32, tag="w",
                                    name=f"mean{j}_{h}")
                    nc.tensor.matmul(mean[:], bd[:], qkr[h][:],
                                     start=True, stop=True)
                    qc[h] = wk.tile([128, 512], BF16, tag="qc",
                                    name=f"qc{j}_{h}")
                    nc.vector.tensor_sub(qc[h][:], qkr[h][:], mean[:])
                    qc2[h] = wk.tile([128, 512], BF16, tag="qc2",
                                     name=f"qc2{j}_{h}")
                    nc.vector.tensor_mul(qc2[h][:], qc[h][:], qc[h][:])
                yield
                # single [128,1024] SBUF lnv so one merged rstd exp suffices
                lnv = wk.tile([128, 1024], F32, tag="lnv", name=f"lnv{j}")
                for h in (0, 1):
                    ssq = wps.tile([128, 512], F32, tag="w",
                                   name=f"ssq{j}_{h}")
                    nc.tensor.matmul(ssq[:], bd[:], qc2[h][:],
                                     start=True, stop=True)
                    nc.scalar.activation(lnv[:, h * 512:(h + 1) * 512],
                                         ssq[:], LN)
                yield
                rstd = wk.tile([128, 1024], BF16, tag="rstd", name=f"rstd{j}")
                nc.scalar.activation(rstd[:], lnv[:], EXP, scale=-0.5,
                                     bias=ddof_b[:])
                for h in (0, 1):
                    sl = slice(h * 512, (h + 1) * 512)
                    nc.vector.tensor_mul(qn[:, j, sl], qc[h][:], rstd[:, sl])
                emit_pad()
                yield

            def v_gen():
                """V token-major + ones cols already in vo. 8 PE pieces."""
                for t in range(TC):
                    vp = wps.tile([128, 512], F32, tag="w", name=f"vp{t}")
                    for k in range(KC):
                        nc.tensor.matmul(
                            vp[:, 0:C],
                            xt[:, k, t * 128:(t + 1) * 128],
                            wq[:, k, 2 * C:3 * C],
                            start=(k == 0), stop=(k == KC - 1))
                    nc.vector.tensor_copy(
                        vo[:, t, :, 0:D],
                        vp[:, 0:C].rearrange("p (h d) -> p h d", h=H))
                    yield

            def av_head_gen(p, es_list, g):
                """AV for head 2p+g on the avps tile: accumulate over the 8
                k-token chunks; rows 64-127 collect the softmax denominator
                broadcast over 64 partitions by the ones-cols. 4 pieces,
                then 1/denom = exp(-ln d) and the normalize mul."""
                av = avps.tile([128, N], F32, tag="av")
                for tp in range(4):
                    for t in (2 * tp, 2 * tp + 1):
                        es = es_list[2 * t + g]
                        for h5 in (0, 1):
                            nc.tensor.matmul(
                                av[:, h5 * 512:(h5 + 1) * 512],
                                vo[:, t, 2 * p + g, :],
                                es[:, h5 * 512:(h5 + 1) * 512],
                                start=(t == 0), stop=(t == TC - 1))
                    yield
                lnd = recp.tile([64, N], F32, tag="lnd")
                nc.scalar.activation(lnd[:], av[64:128, :], LN)
                rec = recp.tile([64, N], F32, tag="rec")
                nc.scalar.activation(rec[:], lnd[:], EXP, scale=-1.0)
                nc.vector.tensor_mul(aoT[64 * g:64 * (g + 1), p, :],
                                     av[0:64, :], rec[:])

            def av_gen(p, es_list):
                yield from av_head_gen(p, es_list, 0)
                yield from av_head_gen(p, es_list, 1)

            tail = {}

            def av_tail_avps_gen(p, es_list, g):
                """Pair-2 head on the avps tile, self-pacing behind the es
                tiles pair_emit is still producing. Denominator handled in
                the explicit tail."""
                av = None
                for tp in range(4):
                    while len(es_list) < 4 * tp + 3:
                        emit_pad()
                        yield
                    if av is None:
                        av = avps.tile([128, N], F32, tag="av")
                        tail["av"] = av
                    for t in (2 * tp, 2 * tp + 1):
                        es = es_list[2 * t + g]
                        for h5 in (0, 1):
                            nc.tensor.matmul(
                                av[:, h5 * 512:(h5 + 1) * 512],
                                vo[:, t, 2 * p + g, :],
                                es[:, h5 * 512:(h5 + 1) * 512],
                                start=(t == 0), stop=(t == TC - 1))
                    yield

            def av_tail_wps_gen(p, es_list, g):
                """Pair-2 second head accumulated in two wps half-tiles so
                it runs concurrently with the avps head instead of after."""
                avh = None
                for tp in range(4):
                    while len(es_list) < 4 * tp + 4:
                        emit_pad()
                        yield
                    if avh is None:
                        wps_free[0] = False
                        avh = [wps.tile([128, 512], F32, tag="w",
                                        name=f"avt{h5}") for h5 in (0, 1)]
                        tail["avh"] = avh
                    for t in (2 * tp, 2 * tp + 1):
                        es = es_list[2 * t + g]
                        for h5 in (0, 1):
                            nc.tensor.matmul(
                                avh[h5][:],
                                vo[:, t, 2 * p + g, :],
                                es[:, h5 * 512:(h5 + 1) * 512],
                                start=(t == 0), stop=(t == TC - 1))
                    yield

            def proj01_gen():
                """proj partial sums over k=0,1 (pairs 0,1 outputs), bias
                folded into the bf16 staging add. 6 PE pieces."""
                for co in range(KC):
                    for h in (0, 1):
                        pf = wps.tile([128, 512], F32, tag="w",
                                      name=f"p01_{co}_{h}")
                        for k in (0, 1):
                            nc.tensor.matmul(
                                pf[:],
                                wp[:, k, co * 128:(co + 1) * 128],
                                aoT[:, k, h * 512:(h + 1) * 512],
                                start=(k == 0), stop=(k == 1))
                        nc.vector.tensor_scalar_add(
                            pstage[:, co, h * 512:(h + 1) * 512],
                            pf[:], pbt[:, co:co + 1])
                        yield

            def pair_emit(p, filler, es_out):
                """Scores + exp for pair p. Per t-chunk the two heads run on
                PE row groups 0/64 (h5-interleaved so they stream
                concurrently); the g-stagger ping-pongs the two s tiles so
                exp(t,g0) overlaps QK(t,g1). 3 filler pieces per t-slot."""
                for t in range(TC):
                    s = {0: sps.tile([128, N], F32, tag="s", name="s0"),
                         1: sps.tile([128, N], F32, tag="s", name="s1")}

                    def qk_mm(g, h5):
                        nc.tensor.matmul(
                            s[g][:, h5 * 512:(h5 + 1) * 512],
                            qn[64 * g:64 * (g + 1), 3 + p,
                               t * 128:(t + 1) * 128],
                            qn[64 * g:64 * (g + 1), p,
                               h5 * 512:(h5 + 1) * 512],
                            start=True, stop=True)

                    qk_mm(0, 0)
                    qk_mm(1, 0)
                    qk_mm(0, 1)
                    es0_t = esp.tile([128, N], BF16, tag="es", name="esg0")
                    nc.scalar.activation(es0_t[:], s[0][:], EXP, scale=scale)
                    qk_mm(1, 1)
                    es1_t = esp.tile([128, N], BF16, tag="es", name="esg1")
                    nc.scalar.activation(es1_t[:], s[1][:], EXP, scale=scale)
                    es_out.append(es0_t)
                    es_out.append(es1_t)
                    for _ in range(3):
                        if next(filler, StopIteration) is StopIteration:
                            emit_pad()

            def chain(*gens):
                for gen in gens:
                    yield from gen

            def spaced1(gen):
                for _ in gen:
                    yield
                    yield

            def interleave(*gens):
                gens = [iter(gg) for gg in gens]
                alive = True
                while alive:
                    alive = False
                    for gg in gens:
                        if next(gg, StopIteration) is not StopIteration:
                            alive = True
                            yield

            def roundrobin_pad(*lanes):
                """Visit lanes cyclically, one piece per visit; an exhausted
                lane contributes a keep-warm pad while any lane remains."""
                lanes = [iter(x) for x in lanes]
                alive = set(range(len(lanes)))
                while alive:
                    for idx in range(len(lanes)):
                        if idx in alive:
                            if next(lanes[idx], StopIteration) is StopIteration:
                                alive.discard(idx)
                                if not alive:
                                    return
                                emit_pad()
                        else:
                            emit_pad()
                        yield

            # ---- schedule ----
            for _ in qkv_chunk_gen(0):
                emit_pad()
            for _ in qkv_chunk_gen(3):
                emit_pad()

            es0, es1, es2 = [], [], []
            pair_emit(0, roundrobin_pad(
                chain(qkv_chunk_gen(1), qkv_chunk_gen(4)), v_gen()), es0)
            pair_emit(1, roundrobin_pad(
                av_gen(0, es0),
                chain(qkv_chunk_gen(2), qkv_chunk_gen(5))), es1)
            # phase E must stay sequential: proj01 reads aoT[:,1,:] which
            # av_gen(1) writes, and both tail accumulators reuse buffers
            # whose previous readers av_gen(1)/proj01 emit.
            filler_e = chain(av_gen(1, es1), proj01_gen(),
                             interleave(av_tail_avps_gen(2, es2, 0),
                                        av_tail_wps_gen(2, es2, 1)))
            pair_emit(2, filler_e, es2)
            for _ in filler_e:
                pass

            # ---- explicit tail: pair-2 normalizers + proj k=2 ----
            av, avh = tail["av"], tail["avh"]
            # head 4 (g0, avps): full-width denominator chain first
            lnd0 = recp.tile([64, N], F32, tag="lnd")
            nc.scalar.activation(lnd0[:], av[64:128, :], LN)
            rec0 = recp.tile([64, N], F32, tag="rec")
            nc.scalar.activation(rec0[:], lnd0[:], EXP, scale=-1.0)
            nc.vector.tensor_mul(aoT[0:64, 2, :], av[0:64, :], rec0[:])
            # head 5 (g1, wps halves) on ACT while g0's mul + proj run
            recs = []
            for h5 in (0, 1):
                lndh = recp.tile([64, 512], F32, tag="lnd2", name=f"lndh{h5}")
                nc.scalar.activation(lndh[:], avh[h5][64:128, :], LN)
                rech = recp.tile([64, 512], F32, tag="rec2", name=f"rech{h5}")
                nc.scalar.activation(rech[:], lndh[:], EXP, scale=-1.0)
                recs.append(rech)
            # proj k=2: co=0 and co=1 start their head-4 rows now (64-row
            # contraction, overlapping head 5's ACT chain); co=2 runs
            # full-width after, reusing co0's sps buffer once po0 is out.
            pf0 = sps.tile([128, N], F32, tag="s", name="pf0")
            pf1 = sps.tile([128, N], F32, tag="s", name="pf1")
            for pf_, co in ((pf0, 0), (pf1, 1)):
                for h5 in (0, 1):
                    nc.tensor.matmul(
                        pf_[:, h5 * 512:(h5 + 1) * 512],
                        wp[0:64, 2, co * 128:(co + 1) * 128],
                        aoT[0:64, 2, h5 * 512:(h5 + 1) * 512],
                        start=True, stop=False)
            for h5 in (0, 1):
                sl = slice(h5 * 512, (h5 + 1) * 512)
                nc.vector.tensor_mul(aoT[64:128, 2, sl],
                                     avh[h5][0:64, :], recs[h5][:])
            for pf_, co in ((pf0, 0), (pf1, 1)):
                for h5 in (0, 1):
                    sl = slice(h5 * 512, (h5 + 1) * 512)
                    nc.tensor.matmul(
                        pf_[:, sl], wp[64:128, 2, co * 128:(co + 1) * 128],
                        aoT[64:128, 2, sl],
                        start=False, stop=False)
                    nc.tensor.matmul(
                        pf_[:, sl], ident[:], pstage[:, co, sl],
                        start=False, stop=True)
            for pf_, co in ((pf0, 0), (pf1, 1)):
                po = pop.tile([128, N], F32, tag="po", name=f"po{co}")
                for h5 in (0, 1):
                    sl = slice(h5 * 512, (h5 + 1) * 512)
                    if (co + h5) % 2 == 0:
                        nc.scalar.copy(po[:, sl], pf_[:, sl])
                    else:
                        nc.vector.tensor_copy(po[:, sl], pf_[:, sl])
                    nc.sync.dma_start(out=outT[co * 128:(co + 1) * 128, sl],
                                      in_=po[:, sl])
            pf2 = sps.tile([128, N], F32, tag="s", name="pf2")
            for h5 in (0, 1):
                sl = slice(h5 * 512, (h5 + 1) * 512)
                nc.tensor.matmul(
                    pf2[:, sl], wp[:, 2, 256:384], aoT[:, 2, sl],
                    start=True, stop=False)
                nc.tensor.matmul(
                    pf2[:, sl], ident[:], pstage[:, 2, sl],
                    start=False, stop=True)
            po2 = pop.tile([128, N], F32, tag="po", name="po2")
            for h5 in (0, 1):
                sl = slice(h5 * 512, (h5 + 1) * 512)
                if h5 == 0:
                    nc.scalar.copy(po2[:, sl], pf2[:, sl])
                else:
                    nc.vector.tensor_copy(po2[:, sl], pf2[:, sl])
                nc.sync.dma_start(out=outT[256:384, sl], in_=po2[:, sl])

    return nc


def _get_prog():
    global _prog
    if _prog is None:
        _prog = _build()
    return _prog


def _make_in_maps(x, qkv_w, proj_w, proj_b):
    import ml_dtypes

    bf16 = ml_dtypes.bfloat16
    qkv_wT = np.ascontiguousarray(np.asarray(qkv_w, np.float32).T.astype(bf16))
    proj_wT = np.ascontiguousarray(np.asarray(proj_w, np.float32).T.astype(bf16))
    pb = np.ascontiguousarray(
        np.asarray(proj_b, np.float32).reshape(3, 128).T)
    bd = np.zeros((128, 128), np.float32)
    for b0 in (0, 64):
        bd[b0:b0 + 64, b0:b0 + 64] = 1.0 / D   # ddof fix in rstd exp bias
    bd = bd.astype(bf16)
    ident = np.eye(128, dtype=np.float32).astype(bf16)

    shared = {"qkv_wT": qkv_wT, "proj_wT": proj_wT, "pb": pb, "bd": bd,
              "ident": ident}
    x = np.asarray(x, np.float32)
    return [
        {"xT": np.ascontiguousarray(x[b].T.astype(bf16)), **shared}
        for b in range(B)
    ]


def run(x, qkv_w, proj_w, proj_b, trace=False):
    from concourse.bass_utils import run_bass_kernel_spmd

    nc = _get_prog()
    in_maps = _make_in_maps(x, qkv_w, proj_w, proj_b)
    res = run_bass_kernel_spmd(
        nc, in_maps, core_ids=list(range(NCORES)), trace=trace)
    out = np.stack([res.results[b]["outT"].T for b in range(B)])
    return np.ascontiguousarray(out.astype(np.float32)), res


def kernel(x, qkv_w, proj_w, proj_b):
    out, _ = run(x, qkv_w, proj_w, proj_b)
    return out
